# revision 13
# baseline (speedup 1.0000x reference)
"""Segment softmax-attention reduce (NicheAttention) on 8 trn2 NeuronCores.

Math (per reference):
    scores = x @ w + b                       [N]
    attn   = scatter_softmax(scores, batch)  [N]   (segments are sorted)
    out    = segment_sum(x * attn[:, None])  [B, D]

Strategy:
  - Segments are partitioned contiguously across 8 cores (rows found via
    searchsorted on the sorted `batch`), rows padded to a common tile count.
  - Per 128-row tile the kernel computes scores with a DVE dot-product
    (tensor_tensor_reduce), e = exp(scores) on ACT, builds a one-hot x e
    matrix A[r, j] = (bmod[r] == j) * e[r] on Pool, and accumulates
    A.T @ [x | 1] into PSUM on PE.  Column D of the output doubles as the
    per-segment denominator (sum of e).
  - The softmax max-shift is skipped: scores ~ N(0, 1), exp() is far inside
    the fp32 range, and softmax is shift-invariant.
  - PSUM rows address segments relative to the first segment of a fixed
    flush window (F tiles); PSUM is flushed to DRAM once per window and the
    host reassembles windows into the global [B, D+1] accumulator, then
    normalizes by the denominator column.
"""

import os

import numpy as np

P = 128
NC = 8
SENTINEL = 200.0

TRACE = os.environ.get("KERNEL_TRACE", "0") == "1"
MM_DTYPE = os.environ.get("KERNEL_MM_DTYPE", "float32")
XBUFS = int(os.environ.get("KERNEL_XBUFS", "12"))
EXP_BATCH = int(os.environ.get("KERNEL_EXP_BATCH", "1"))
DMA_SPLIT = os.environ.get("KERNEL_DMA_SPLIT", "0") == "1"
NO_COMPUTE = os.environ.get("KERNEL_NO_COMPUTE", "0") == "1"
LAST_RESULT = None


def _build(nc, Nt, F, Nf, D, bias, repeat=1):
    from contextlib import ExitStack

    from concourse import mybir, tile

    f32 = mybir.dt.float32
    mm_dt = getattr(mybir.dt, MM_DTYPE)

    xe_d = nc.declare_dram_parameter("xe", [Nt * P, D + 1], f32, isOutput=False)
    bmod_d = nc.declare_dram_parameter("bmod", [P, Nt], f32, isOutput=False)
    wb_d = nc.declare_dram_parameter("wb", [P, D], f32, isOutput=False)
    iota_d = nc.declare_dram_parameter("iota", [P, P], f32, isOutput=False)
    out_d = nc.declare_dram_parameter("out", [Nf, P, D + 1], f32, isOutput=True)

    with tile.TileContext(nc) as tc, ExitStack() as ctx:
        const_pool = ctx.enter_context(tc.tile_pool(name="const", bufs=1))
        xpool = ctx.enter_context(tc.tile_pool(name="xp", bufs=XBUFS))
        apool = ctx.enter_context(tc.tile_pool(name="ap", bufs=4))
        opool = ctx.enter_context(tc.tile_pool(name="op", bufs=2))
        psum_pool = ctx.enter_context(tc.tile_pool(name="ps", bufs=2, space="PSUM"))

        wb_t = const_pool.tile([P, D], f32)
        nc.sync.dma_start(wb_t[:], wb_d[:])
        iota_t = const_pool.tile([P, P], f32)
        nc.sync.dma_start(iota_t[:], iota_d[:])
        bmod_t = const_pool.tile([P, Nt], f32)
        nc.sync.dma_start(bmod_t[:], bmod_d[:])
        scores_t = const_pool.tile([P, Nt], f32)
        e_t = const_pool.tile([P, Nt], f32)
        scratch = const_pool.tile([P, D], f32)
        bias_t = const_pool.tile([P, 1], f32)
        nc.vector.memset(bias_t[:], bias)

        def mm(ap):
            return ap if mm_dt == f32 else ap.bitcast(mm_dt)

        for _rep in range(repeat):
            for wi in range(Nf):
                t_lo = wi * F
                t_hi = min((wi + 1) * F, Nt)
                if NO_COMPUTE:
                    last_xt = None
                    for t in range(t_lo, t_hi):
                        xt = xpool.tile([P, D + 1], f32)
                        eng = nc.scalar if (DMA_SPLIT and t % 2) else nc.sync
                        eng.dma_start(xt[:], xe_d[t * P : (t + 1) * P, :])
                        last_xt = xt
                    nc.sync.dma_start(out_d[wi], last_xt[:])
                    continue
                psum_t = psum_pool.tile([P, D + 1], f32)
                for g_lo in range(t_lo, t_hi, EXP_BATCH):
                    g_hi = min(g_lo + EXP_BATCH, t_hi)
                    xts = []
                    for t in range(g_lo, g_hi):
                        xt = xpool.tile([P, D + 1], f32)
                        eng = nc.scalar if (DMA_SPLIT and t % 2) else nc.sync
                        eng.dma_start(xt[:], xe_d[t * P : (t + 1) * P, :])
                        xts.append(xt)
                        nc.vector.scalar_tensor_tensor(
                            out=scratch[:],
                            in0=xt[:, 0:D],
                            scalar=1.0,
                            in1=wb_t[:],
                            op0=mybir.AluOpType.mult,
                            op1=mybir.AluOpType.mult,
                            accum_out=scores_t[:, t : t + 1],
                        )
                    nc.scalar.activation(
                        e_t[:, g_lo:g_hi],
                        scores_t[:, g_lo:g_hi],
                        mybir.ActivationFunctionType.Exp,
                        bias=bias_t[:],
                    )
                    for t in range(g_lo, g_hi):
                        xt = xts[t - g_lo]
                        A_t = apool.tile([P, P], f32)
                        nc.gpsimd.tensor_scalar(
                            out=A_t[:],
                            in0=iota_t[:],
                            scalar1=bmod_t[:, t : t + 1],
                            scalar2=e_t[:, t : t + 1],
                            op0=mybir.AluOpType.is_equal,
                            op1=mybir.AluOpType.mult,
                        )
                        nc.tensor.matmul(
                            psum_t[:],
                            lhsT=mm(A_t[:]),
                            rhs=mm(xt[:]),
                            start=(t == t_lo),
                            stop=(t == t_hi - 1),
                        )
                ot = opool.tile([P, D + 1], f32)
                nc.scalar.copy(ot[:], psum_t[:])
                nc.scalar.dma_start(out_d[wi], ot[:])


def _prepare(x, w, b, batch, num_segments):
    x = np.asarray(x, dtype=np.float32)
    w = np.asarray(w, dtype=np.float32)
    bias = float(np.asarray(b))
    batch_i = np.asarray(batch).astype(np.int64)
    B = int(np.asarray(num_segments))
    N, D = x.shape

    segs_per_core = -(-B // NC)
    seg_bounds = np.minimum(np.arange(NC + 1) * segs_per_core, B)
    row_bounds = np.searchsorted(batch_i, seg_bounds, side="left")
    Rc = np.diff(row_bounds)
    Nt = max(1, int(-(-int(Rc.max()) // P)))

    # Pick the largest flush window F (in tiles) such that every window of
    # 128*F rows on every core spans < 128 distinct (consecutive) segments.
    F = 64
    while True:
        ok = True
        for c in range(NC):
            lo, hi = int(row_bounds[c]), int(row_bounds[c + 1])
            for r0 in range(0, hi - lo, F * P):
                r1 = min(r0 + F * P, hi - lo)
                if batch_i[lo + r1 - 1] - batch_i[lo + r0] > P - 1:
                    ok = False
                    break
            if not ok:
                break
        if ok:
            break
        F //= 2
        assert F >= 1, "single-tile window still spans >=128 segments"
    Nf = -(-Nt // F)

    wb = np.ascontiguousarray(np.broadcast_to(w, (P, D)))
    iota_np = np.ascontiguousarray(
        np.broadcast_to(np.arange(P, dtype=np.float32), (P, P))
    )

    in_maps = []
    smin_list = []
    for c in range(NC):
        lo, hi = int(row_bounds[c]), int(row_bounds[c + 1])
        R = hi - lo
        xe = np.zeros((Nt * P, D + 1), dtype=np.float32)
        xe[:R, :D] = x[lo:hi]
        xe[:R, D] = 1.0
        bm = np.full(Nt * P, SENTINEL, dtype=np.float32)
        smins = np.full(Nf, -1, dtype=np.int64)
        for wi in range(Nf):
            r0 = wi * F * P
            r1 = min(r0 + F * P, R)
            if r0 >= R:
                break
            s0 = int(batch_i[lo + r0])
            smins[wi] = s0
            bm[r0:r1] = (batch_i[lo + r0 : lo + r1] - s0).astype(np.float32)
        bmod = np.ascontiguousarray(bm.reshape(Nt, P).T)
        in_maps.append({"xe": xe, "bmod": bmod, "wb": wb, "iota": iota_np})
        smin_list.append(smins)

    return {
        "in_maps": in_maps,
        "smin_list": smin_list,
        "Nt": Nt,
        "F": F,
        "Nf": Nf,
        "D": D,
        "B": B,
        "bias": bias,
    }


def _reassemble(results, meta):
    B, D, Nf = meta["B"], meta["D"], meta["Nf"]
    acc = np.zeros((B, D + 1), dtype=np.float64)
    for c in range(NC):
        out_c = results[c]["out"]
        smins = meta["smin_list"][c]
        for wi in range(Nf):
            s0 = int(smins[wi])
            if s0 < 0:
                continue
            hi_s = min(s0 + P, B)
            acc[s0:hi_s] += out_c[wi, : hi_s - s0].astype(np.float64)

    denom = acc[:, D]
    with np.errstate(divide="ignore", invalid="ignore"):
        out = np.where(denom[:, None] > 0, acc[:, :D] / denom[:, None], 0.0)
    return out.astype(np.float32)


def kernel(x, w, b, batch, num_segments):
    global LAST_RESULT

    from concourse import bacc, bass_utils

    meta = _prepare(x, w, b, batch, num_segments)
    nc = bacc.Bacc("TRN2", debug=False)
    _build(nc, meta["Nt"], meta["F"], meta["Nf"], meta["D"], meta["bias"])
    nc.compile()
    rb = bass_utils.run_bass_kernel_spmd(
        nc, meta["in_maps"], list(range(NC)), trace=TRACE
    )
    LAST_RESULT = rb
    return _reassemble(rb.results, meta)
